# revision 1
# baseline (speedup 1.0000x reference)
"""CvT attention block (depthwise conv proj + BN + talking-heads attention) on 8 trn2 cores.

Sharding: data-parallel over batch (16 batches -> 2 per core). BN batch stats are
reduced across cores with a single small AllReduce (6x192 floats).

Layouts/folds (all matmuls bf16, fp32 PSUM accumulation):
  - host passes inputs channel-major [b, c, h*w]; depthwise conv runs as 9
    diagonal-matmul taps over zero-padded [c, 58*58] tiles.
  - BN (A = scale*rsqrt(var+eps), B = offset - mean*A) folds into the pointwise
    weights; the bias enters through an appended ones-row on the contraction dim.
  - pre-softmax talking heads fold into K's pointwise weights (3 expanded K_g).
  - post-softmax talking heads AND the output projection w_out fold into V's
    weights: VW_g = BN(conv_v) @ (pw_v @ (post_sm[g,:]-scaled w_out)).
  - scores are built transposed [k_pos, l]; exp on ScalarE with no max-subtract
    (logits are ~1e-1); softmax denominator Z arrives as a ones-column through
    the AV matmul; the final output is Sum_g U_g[:, :192] * (1/Z_g[l]).
"""

import os
import sys
import functools

sys.path.insert(0, "/opt/trn_rl_repo")
os.environ.setdefault("MYCRO_LOCAL_CACHE", "1")

import numpy as np
import ml_dtypes

import concourse.bass as bass
import concourse.mybir as mybir
import concourse.tile as tile
from concourse import bacc
from concourse.bass_utils import run_bass_kernel_spmd

F32 = mybir.dt.float32
BF16 = mybir.dt.bfloat16
AF = mybir.ActivationFunctionType
ALU = mybir.AluOpType
AX = mybir.AxisListType

N_CORES = 8
BPC = 2                      # batches per core
C = 192                      # channels
HD = 192                     # num_heads * head_ch
NH = 3
LQ = 3136                    # 56*56
LK = 784                     # 28*28
PADW = 58
PADN = PADW * PADW           # 3364
EPS = 1e-5

CT = [(0, 128), (128, 64)]   # channel tiles (partition dim)
KT112 = [(i * 112, 112) for i in range(7)]                # k_pos tiles
LCH = [(i * 512, 512) for i in range(6)] + [(3072, 64)]   # l chunks

last_results = None


def _emit(tc, nc, io, n_cores=N_CORES, mock_cc=False, phases=3):
    xq, xkv, dwt, vecs_d, pwq_d, pwk_d, pwvT_d, wout_d, sm_d, out_d, cc_in, cc_out = io
    ident_d = nc.inline_tensor(np.eye(128, dtype=ml_dtypes.bfloat16), name="ident128")
    MS = bass.MemorySpace

    with tc.tile_pool(name="wpool", bufs=1) as W, \
         tc.tile_pool(name="ypool", bufs=1) as Y:

        # ---------- static loads ----------
        ident = W.tile([128, 128], BF16, tag="ident")
        nc.sync.dma_start(ident[:, :], ident_d.ap())

        dw, vecs = [], []
        for ci, (c0, csz) in enumerate(CT):
            t = W.tile([csz, 27], F32, tag=f"dw{ci}", name=f"dw{ci}")
            nc.sync.dma_start(t[:, :], dwt.ap()[c0:c0 + csz, :])
            dw.append(t)
            t = W.tile([csz, 6], F32, tag=f"vecs{ci}", name=f"vecs{ci}")
            nc.sync.dma_start(t[:, :], vecs_d.ap()[c0:c0 + csz, :])
            vecs.append(t)
        smrow = W.tile([1, 18], F32, tag="smrow")
        nc.sync.dma_start(smrow[:, :], sm_d.ap()[:, :])
        smbc = W.tile([128, 18], F32, tag="smbc")
        nc.gpsimd.partition_broadcast(smbc[:, :], smrow[:, :])

        pwq_sb, pwk_sb, pwvT_sb, wout_sb = [], [], [], []
        for ci, (c0, csz) in enumerate(CT):
            for lst, dram, nm in ((pwq_sb, pwq_d, "pwq"), (pwk_sb, pwk_d, "pwk"),
                                  (pwvT_sb, pwvT_d, "pwvT"), (wout_sb, wout_d, "wout")):
                t = W.tile([csz, 192], F32, tag=f"{nm}{ci}", name=f"{nm}{ci}")
                nc.sync.dma_start(t[:, :], dram.ap()[c0:c0 + csz, :])
                lst.append(t)

        diag = {}
        for p in range(3):  # 0=q 1=k 2=v
            for ci, (c0, csz) in enumerate(CT):
                t = W.tile([csz, 9, csz], BF16, tag=f"diag{p}{ci}", name=f"diag{p}{ci}")
                for tp in range(9):
                    nc.vector.tensor_scalar(t[:, tp, :], ident[0:csz, 0:csz],
                                            dw[ci][:, 9 * p + tp:9 * p + tp + 1], None, ALU.mult)
                diag[(p, ci)] = t

        # conv outputs (augmented with ones row on tile 2)
        ysz = {0: LQ, 1: LK, 2: LK}
        y = {}
        for b in range(BPC):
            for p in range(3):
                y[(b, p, 0)] = Y.tile([128, ysz[p]], BF16, tag=f"y{b}{p}0", name=f"y{b}{p}0")
                y[(b, p, 1)] = Y.tile([65, ysz[p]], BF16, tag=f"y{b}{p}1", name=f"y{b}{p}1")
                nc.gpsimd.memset(y[(b, p, 1)][64:65, :], 1.0)

        # bn_stats slots: 22 groups of 6 (q: 0..13, k: 14..17, v: 18..21)
        slots = [W.tile([csz, 132], F32, tag=f"slots{ci}", name=f"slots{ci}")
                 for ci, (c0, csz) in enumerate(CT)]
        ccst = [W.tile([csz, 6], F32, tag=f"ccst{ci}", name=f"ccst{ci}")
                for ci, (c0, csz) in enumerate(CT)]
        gst = [W.tile([csz, 6], F32, tag=f"gst{ci}", name=f"gst{ci}")
               for ci, (c0, csz) in enumerate(CT)]

        # ---------- phase 1: stage, pad, conv, stats ----------
        with tc.tile_pool(name="stpool", bufs=4) as ST, \
             tc.tile_pool(name="xppool", bufs=2) as XP, \
             tc.tile_pool(name="pconv", bufs=3, space=MS.PSUM) as PCONV:

            for b in range(BPC):
                xpad = {}
                for inp in range(2):  # 0 = xq, 1 = xkv
                    src = xq if inp == 0 else xkv
                    for ci, (c0, csz) in enumerate(CT):
                        xp = XP.tile([csz, PADN], BF16, tag=f"xp{inp}{ci}", name=f"xp{inp}{ci}")
                        nc.gpsimd.memset(xp[:, :], 0.0)
                        for ch in range(7):  # 7 chunks of 448 = 8 rows of 56
                            stg = ST.tile([128, 448], F32, tag="stage", name="stage")
                            nc.sync.dma_start(stg[0:csz, :],
                                              src.ap()[b, c0:c0 + csz, 448 * ch:448 * (ch + 1)])
                            dst = xp.rearrange("p (h w) -> p h w", w=PADW)
                            r0 = 8 * ch + 1
                            nc.vector.tensor_copy(
                                dst[0:csz, r0:r0 + 8, 1:57],
                                stg.rearrange("p (h w) -> p h w", w=56)[0:csz, :, :])
                        xpad[(inp, ci)] = xp

                # q conv: stride 1, psum chunks of 448 (8 output rows)
                for ci, (c0, csz) in enumerate(CT):
                    xv = xpad[(0, ci)].rearrange("p (h w) -> p h w", w=PADW)
                    for qc in range(7):
                        ps = PCONV.tile([csz, 448], F32, tag="convps", name="convps")
                        t = 0
                        for dy in (-1, 0, 1):
                            for dx in (-1, 0, 1):
                                r0 = 8 * qc + 1 + dy
                                rhs = xv[0:csz, r0:r0 + 8, 1 + dx:57 + dx]
                                nc.tensor.matmul(ps[:, :], diag[(0, ci)][:, t, :], rhs,
                                                 start=(t == 0), stop=(t == 8))
                                t += 1
                        si = 7 * b + qc
                        ysl = y[(b, 0, ci)][0:csz, 448 * qc:448 * (qc + 1)]
                        nc.scalar.activation(ysl, ps[:, :], AF.Copy)
                        nc.vector.bn_stats(slots[ci][:, 6 * si:6 * si + 6], ysl)

                # k/v conv: stride 2 over xkv, psum chunks of 392 (14 output rows)
                for p in (1, 2):
                    for ci, (c0, csz) in enumerate(CT):
                        xv = xpad[(1, ci)].rearrange("p (h th w tw) -> p h th w tw",
                                                     th=2, tw=2, w=29)
                        for kc in range(2):
                            ps = PCONV.tile([csz, 392], F32, tag="convps", name="convps")
                            t = 0
                            for dy in (0, 1, 2):
                                for dx in (0, 1, 2):
                                    h0, th = divmod(28 * kc + dy + 1, 2)
                                    w0, tw = divmod(dx + 1, 2)
                                    rhs = xv[0:csz, h0:h0 + 14, th, w0:w0 + 28, tw]
                                    nc.tensor.matmul(ps[:, :], diag[(p, ci)][:, t, :], rhs,
                                                     start=(t == 0), stop=(t == 8))
                                    t += 1
                            si = 14 + (p - 1) * 4 + 2 * b + kc
                            ysl = y[(b, p, ci)][0:csz, 392 * kc:392 * (kc + 1)]
                            nc.scalar.activation(ysl, ps[:, :], AF.Copy)
                            nc.vector.bn_stats(slots[ci][:, 6 * si:6 * si + 6], ysl)

        # ---------- phase 2: AllReduce of stats, A/B vectors, weight folds ----------
        if phases < 2:
            return
        # local (mean, var) per path via bn_aggr, converted to (sum, sumsq) for AllReduce
        NLOC = {0: float(BPC * LQ), 1: float(BPC * LK), 2: float(BPC * LK)}
        mv = [W.tile([csz, 6], F32, tag=f"mv{ci}", name=f"mv{ci}")
              for ci, (c0, csz) in enumerate(CT)]
        msq = [W.tile([csz, 3], F32, tag=f"msq{ci}", name=f"msq{ci}")
               for ci, (c0, csz) in enumerate(CT)]
        for ci, (c0, csz) in enumerate(CT):
            for p, (a0, a1) in enumerate(((0, 84), (84, 108), (108, 132))):
                nc.vector.bn_aggr(mv[ci][:, 2 * p:2 * p + 2], slots[ci][:, a0:a1])
                m = mv[ci][:, 2 * p:2 * p + 1]
                v = mv[ci][:, 2 * p + 1:2 * p + 2]
                nc.vector.tensor_scalar(ccst[ci][:, 2 * p:2 * p + 1], m, NLOC[p], None, ALU.mult)
                nc.vector.tensor_scalar(msq[ci][:, p:p + 1], m, m, None, ALU.mult)
                nc.vector.tensor_scalar(ccst[ci][:, 2 * p + 1:2 * p + 2], v,
                                        msq[ci][:, p:p + 1], NLOC[p], ALU.add, ALU.mult)
            nc.gpsimd.dma_start(cc_in.ap()[c0:c0 + csz, :], ccst[ci][:, :])

        if mock_cc:
            nc.gpsimd.dma_start(cc_out.ap()[:, :], cc_in.ap()[:, :])
        else:
            nc.gpsimd.collective_compute(
                "AllReduce", ALU.add, replica_groups=[list(range(n_cores))],
                ins=[cc_in.ap()], outs=[cc_out.ap()])

        for ci, (c0, csz) in enumerate(CT):
            nc.gpsimd.dma_start(gst[ci][:, :], cc_out.ap()[c0:c0 + csz, :])

        # ab cols: [A_q' 0 | A_k 1 | A_v 2 | mean_q 3 | mean_k 4 | mean_v 5]
        ab = [W.tile([csz, 6], F32, tag=f"ab{ci}", name=f"ab{ci}")
              for ci, (c0, csz) in enumerate(CT)]
        bbf = [W.tile([csz, 3], BF16, tag=f"bbf{ci}", name=f"bbf{ci}")
               for ci, (c0, csz) in enumerate(CT)]
        tmp = [W.tile([csz, 4], F32, tag=f"tmp{ci}", name=f"tmp{ci}")
               for ci, (c0, csz) in enumerate(CT)]
        nb = float(n_cores * BPC)
        NTOT = {0: nb * LQ, 1: nb * LK, 2: nb * LK}
        for ci, (c0, csz) in enumerate(CT):
            for p in range(3):
                s1 = gst[ci][:, 2 * p:2 * p + 1]
                s2 = gst[ci][:, 2 * p + 1:2 * p + 2]
                mean = ab[ci][:, 3 + p:4 + p]
                inv_n = 1.0 / NTOT[p]
                nc.vector.tensor_scalar(mean, s1, inv_n, None, ALU.mult)
                t0 = tmp[ci][:, 0:1]
                nc.vector.tensor_scalar(t0, s2, inv_n, EPS, ALU.mult, ALU.add)
                sq = tmp[ci][:, 1:2]
                nc.vector.tensor_scalar(sq, mean, mean, None, ALU.mult)
                tv = tmp[ci][:, 2:3]
                nc.vector.tensor_tensor(tv, t0, sq, ALU.subtract)
                lt = tmp[ci][:, 3:4]
                nc.scalar.activation(lt, tv, AF.Ln)
                rstd = tmp[ci][:, 0:1]      # reuse
                nc.scalar.activation(rstd, lt, AF.Exp, scale=-0.5)
                A = ab[ci][:, p:p + 1]
                nc.vector.tensor_scalar(A, rstd, vecs[ci][:, 2 * p:2 * p + 1], None, ALU.mult)
                recA = tmp[ci][:, 1:2]      # reuse
                nc.vector.reciprocal(recA, A)
                bpp = tmp[ci][:, 2:3]       # reuse; b'' = offset*recA - mean
                nc.vector.scalar_tensor_tensor(bpp, vecs[ci][:, 2 * p + 1:2 * p + 2], recA,
                                               mean, ALU.mult, ALU.subtract)
                nc.vector.tensor_scalar(bbf[ci][:, p:p + 1], bpp, 1.0, None, ALU.mult)
                if p == 0:
                    nc.vector.tensor_scalar(A, A, 0.125, None, ALU.mult)

        pwqA = [W.tile([128, 192], BF16, tag="pwqA0", name="pwqA0"),
                W.tile([65, 192], BF16, tag="pwqA1", name="pwqA1")]
        pwkA = [W.tile([csz, 192], BF16, tag=f"pwkA{ci}", name=f"pwkA{ci}")
                for ci, (c0, csz) in enumerate(CT)]
        browk = W.tile([1, 192], F32, tag="browk")
        pwvT_bf = [W.tile([csz, 192], BF16, tag=f"pwvTb{ci}", name=f"pwvTb{ci}")
                   for ci, (c0, csz) in enumerate(CT)]
        postvec = W.tile([128, 3], F32, tag="postvec")
        wbar = [W.tile([128, 192], BF16, tag="wbar0", name="wbar0"),
                W.tile([64, 192], BF16, tag="wbar1", name="wbar1")]
        kw, cw = {}, {}
        for g in range(NH):
            kw[(g, 0)] = W.tile([128, 192], BF16, tag=f"kw{g}0", name=f"kw{g}0")
            kw[(g, 1)] = W.tile([65, 192], BF16, tag=f"kw{g}1", name=f"kw{g}1")
            cw[(g, 0)] = W.tile([128, 193], BF16, tag=f"cw{g}0", name=f"cw{g}0")
            cw[(g, 1)] = W.tile([65, 193], BF16, tag=f"cw{g}1", name=f"cw{g}1")

        with tc.tile_pool(name="prow", bufs=2, space=MS.PSUM) as PROW, \
             tc.tile_pool(name="pcw", bufs=2, space=MS.PSUM) as PCW:

            # q weights
            for ci, (c0, csz) in enumerate(CT):
                nc.vector.tensor_scalar(pwqA[ci][0:csz, :], pwq_sb[ci][:, :],
                                        ab[ci][:, 0:1], None, ALU.mult)
            rps = PROW.tile([1, 192], F32, tag="rowps", name="rowps")
            nc.tensor.matmul(rps[:, :], bbf[0][:, 0:1], pwqA[0][0:128, :], start=True, stop=False)
            nc.tensor.matmul(rps[:, :], bbf[1][:, 0:1], pwqA[1][0:64, :], start=False, stop=True)
            nc.vector.tensor_copy(pwqA[1][64:65, :], rps[:, :])

            # k weights + pre_sm folding
            for ci, (c0, csz) in enumerate(CT):
                nc.vector.tensor_scalar(pwkA[ci][:, :], pwk_sb[ci][:, :],
                                        ab[ci][:, 1:2], None, ALU.mult)
            rps = PROW.tile([1, 192], F32, tag="rowps", name="rowps")
            nc.tensor.matmul(rps[:, :], bbf[0][:, 1:2], pwkA[0][:, :], start=True, stop=False)
            nc.tensor.matmul(rps[:, :], bbf[1][:, 1:2], pwkA[1][:, :], start=False, stop=True)
            nc.vector.tensor_copy(browk[:, :], rps[:, :])
            for g in range(NH):
                for h in range(NH):
                    col = 3 * h + g
                    for ci, (c0, csz) in enumerate(CT):
                        nc.vector.tensor_scalar(kw[(g, ci)][0:csz, 64 * h:64 * (h + 1)],
                                                pwkA[ci][:, 64 * h:64 * (h + 1)],
                                                smbc[0:csz, col:col + 1], None, ALU.mult)
                    nc.vector.tensor_scalar(kw[(g, 1)][64:65, 64 * h:64 * (h + 1)],
                                            browk[:, 64 * h:64 * (h + 1)],
                                            smbc[0:1, col:col + 1], None, ALU.mult)

            # v weights: cw_aug_g with post_sm + w_out folded
            for ci, (c0, csz) in enumerate(CT):
                nc.vector.tensor_copy(pwvT_bf[ci][:, :], pwvT_sb[ci][:, :])
            for g in range(NH):
                nc.vector.tensor_copy(postvec[0:64, g:g + 1], smbc[0:64, 9 + 3 * g:10 + 3 * g])
                nc.vector.tensor_copy(postvec[64:128, g:g + 1], smbc[64:128, 10 + 3 * g:11 + 3 * g])
                nc.vector.tensor_scalar(wbar[0][:, :], wout_sb[0][:, :],
                                        postvec[:, g:g + 1], None, ALU.mult)
                nc.vector.tensor_scalar(wbar[1][:, :], wout_sb[1][:, :],
                                        smbc[0:64, 11 + 3 * g:12 + 3 * g], None, ALU.mult)
                for ci, (c0, csz) in enumerate(CT):
                    ps = PCW.tile([csz, 192], F32, tag="cwps", name="cwps")
                    nc.tensor.matmul(ps[:, :], pwvT_bf[0][:, c0:c0 + csz], wbar[0][:, :],
                                     start=True, stop=False)
                    nc.tensor.matmul(ps[:, :], pwvT_bf[1][:, c0:c0 + csz], wbar[1][:, :],
                                     start=False, stop=True)
                    nc.scalar.activation(cw[(g, ci)][0:csz, 0:192], ps[:, :], AF.Copy,
                                         scale=ab[ci][:, 2:3])
                rps = PROW.tile([1, 192], F32, tag="rowps", name="rowps")
                nc.tensor.matmul(rps[:, :], bbf[0][:, 2:3], cw[(g, 0)][0:128, 0:192],
                                 start=True, stop=False)
                nc.tensor.matmul(rps[:, :], bbf[1][:, 2:3], cw[(g, 1)][0:64, 0:192],
                                 start=False, stop=True)
                nc.vector.tensor_copy(cw[(g, 1)][64:65, 0:192], rps[:, :])
                nc.gpsimd.memset(cw[(g, 0)][:, 192:193], 0.0)
                nc.gpsimd.memset(cw[(g, 1)][0:64, 192:193], 0.0)
                nc.gpsimd.memset(cw[(g, 1)][64:65, 192:193], 1.0)

        # ---------- phase 3: projections + attention ----------
        if phases < 3:
            return
        with tc.tile_pool(name="qtpool", bufs=2) as QT, \
             tc.tile_pool(name="ktpool", bufs=2) as KTP, \
             tc.tile_pool(name="vwpool", bufs=2) as VWP, \
             tc.tile_pool(name="epool", bufs=2) as EP, \
             tc.tile_pool(name="accpool", bufs=3) as ACC, \
             tc.tile_pool(name="outpool", bufs=3) as OUT, \
             tc.tile_pool(name="rzpool", bufs=8) as RZ, \
             tc.tile_pool(name="pbig", bufs=3, space=MS.PSUM) as PBIG, \
             tc.tile_pool(name="pav", bufs=4, space=MS.PSUM) as PAV:

            for b in range(BPC):
                qt = [QT.tile([128, LQ], BF16, tag="qt0", name="qt0"),
                      QT.tile([64, LQ], BF16, tag="qt1", name="qt1")]
                for l0, lsz in LCH:
                    for mi, (m0, msz) in enumerate(CT):
                        ps = PBIG.tile([msz, lsz], F32, tag="bigps", name="bigps")
                        nc.tensor.matmul(ps[:, :], pwqA[0][:, m0:m0 + msz],
                                         y[(b, 0, 0)][:, l0:l0 + lsz], start=True, stop=False)
                        nc.tensor.matmul(ps[:, :], pwqA[1][:, m0:m0 + msz],
                                         y[(b, 0, 1)][:, l0:l0 + lsz], start=False, stop=True)
                        nc.vector.tensor_copy(qt[mi][0:msz, l0:l0 + lsz], ps[:, :])

                kt = {}
                for g in range(NH):
                    kt[(g, 0)] = KTP.tile([128, LK], BF16, tag=f"kt{g}0", name=f"kt{g}0")
                    kt[(g, 1)] = KTP.tile([64, LK], BF16, tag=f"kt{g}1", name=f"kt{g}1")
                    for k0 in (0, 392):
                        for mi, (m0, msz) in enumerate(CT):
                            ps = PBIG.tile([msz, 392], F32, tag="bigps", name="bigps")
                            nc.tensor.matmul(ps[:, :], kw[(g, 0)][:, m0:m0 + msz],
                                             y[(b, 1, 0)][:, k0:k0 + 392], start=True, stop=False)
                            nc.tensor.matmul(ps[:, :], kw[(g, 1)][:, m0:m0 + msz],
                                             y[(b, 1, 1)][:, k0:k0 + 392], start=False, stop=True)
                            nc.vector.tensor_copy(kt[(g, mi)][0:msz, k0:k0 + 392], ps[:, :])

                vw = {}
                for g in range(NH):
                    vw[g] = VWP.tile([112, 7, 193], BF16, tag=f"vw{g}", name=f"vw{g}")
                    for ki, (k0, ksz) in enumerate(KT112):
                        ps = PAV.tile([112, 193], F32, tag="avps", name="avps")
                        nc.tensor.matmul(ps[:, :], y[(b, 2, 0)][:, k0:k0 + ksz], cw[(g, 0)][:, :],
                                         start=True, stop=False)
                        nc.tensor.matmul(ps[:, :], y[(b, 2, 1)][:, k0:k0 + ksz], cw[(g, 1)][:, :],
                                         start=False, stop=True)
                        nc.vector.tensor_copy(vw[g][:, ki, :], ps[:, :])

                for l0, lsz in LCH:
                    ee = {}
                    for g in range(NH):
                        ee[g] = EP.tile([112, 7, 512], BF16, tag=f"e{g}", name=f"e{g}")
                        for ki, (k0, ksz) in enumerate(KT112):
                            ps = PBIG.tile([112, lsz], F32, tag="bigps", name="bigps")
                            nc.tensor.matmul(ps[:, :], kt[(g, 0)][:, k0:k0 + ksz],
                                             qt[0][:, l0:l0 + lsz], start=True, stop=False)
                            nc.tensor.matmul(ps[:, :], kt[(g, 1)][:, k0:k0 + ksz],
                                             qt[1][:, l0:l0 + lsz], start=False, stop=True)
                            nc.scalar.activation(ee[g][:, ki, 0:lsz], ps[:, :], AF.Exp)

                    for ls in range(0, lsz, 128):
                        lw = min(128, lsz - ls)
                        ups, rz = [], []
                        for g in range(NH):
                            ps = PAV.tile([lw, 193], F32, tag="avps", name="avps")
                            for ki in range(7):
                                nc.tensor.matmul(ps[:, :], ee[g][:, ki, ls:ls + lw],
                                                 vw[g][:, ki, :],
                                                 start=(ki == 0), stop=(ki == 6))
                            r = RZ.tile([lw, 1], F32, tag="rz", name="rz")
                            nc.vector.reciprocal(r[:, :], ps[:, 192:193])
                            ups.append(ps)
                            rz.append(r)
                        acc = ACC.tile([lw, 192], F32, tag="acc", name="acc")
                        ot = OUT.tile([lw, 192], F32, tag="ot", name="ot")
                        nc.vector.tensor_scalar(acc[:, :], ups[0][:, 0:192], rz[0][:, :],
                                                None, ALU.mult)
                        nc.vector.scalar_tensor_tensor(acc[:, :], ups[1][:, 0:192], rz[1][:, :],
                                                       acc[:, :], ALU.mult, ALU.add)
                        nc.vector.scalar_tensor_tensor(ot[:, :], ups[2][:, 0:192], rz[2][:, :],
                                                       acc[:, :], ALU.mult, ALU.add)
                        nc.sync.dma_start(out_d.ap()[b, l0 + ls:l0 + ls + lw, :], ot[:, :])


def build(n_cores=N_CORES, mock_cc=False, phases=3):
    nc = bacc.Bacc("TRN2", target_bir_lowering=False, debug=False, num_devices=n_cores)
    xq = nc.dram_tensor("xq", [BPC, C, LQ], F32, kind="ExternalInput")
    xkv = nc.dram_tensor("xkv", [BPC, C, LQ], F32, kind="ExternalInput")
    dwt = nc.dram_tensor("dwt", [C, 27], F32, kind="ExternalInput")
    vecs_d = nc.dram_tensor("vecs", [C, 6], F32, kind="ExternalInput")
    pwq_d = nc.dram_tensor("pwq", [C, HD], F32, kind="ExternalInput")
    pwk_d = nc.dram_tensor("pwk", [C, HD], F32, kind="ExternalInput")
    pwvT_d = nc.dram_tensor("pwvT", [HD, C], F32, kind="ExternalInput")
    wout_d = nc.dram_tensor("wout", [HD, C], F32, kind="ExternalInput")
    sm_d = nc.dram_tensor("smm", [1, 18], F32, kind="ExternalInput")
    out_d = nc.dram_tensor("out", [BPC, LQ, C], F32, kind="ExternalOutput")
    cc_in = nc.dram_tensor("cc_in", [C, 6], F32)
    cc_out = nc.dram_tensor("cc_out", [C, 6], F32)

    io = (xq, xkv, dwt, vecs_d, pwq_d, pwk_d, pwvT_d, wout_d, sm_d, out_d, cc_in, cc_out)
    with tile.TileContext(nc) as tc:
        _emit(tc, nc, io, n_cores=n_cores, mock_cc=mock_cc, phases=phases)
    nc.compile()
    return nc


@functools.lru_cache(maxsize=1)
def _built():
    return build()


def make_in_maps(inputs):
    f32c = lambda a: np.ascontiguousarray(np.asarray(a), dtype=np.float32)
    xq_t = f32c(np.asarray(inputs["inputs_q"]).transpose(0, 3, 1, 2).reshape(16, C, LQ))
    xkv_t = f32c(np.asarray(inputs["inputs_kv"]).transpose(0, 3, 1, 2).reshape(16, C, LQ))
    dwt = f32c(np.concatenate([np.asarray(inputs[k]).reshape(9, C).T
                               for k in ("dw_q", "dw_k", "dw_v")], axis=1))
    vecs = f32c(np.stack([np.asarray(inputs[k]) for k in
                          ("scale_q", "offset_q", "scale_k", "offset_k", "scale_v", "offset_v")],
                         axis=1))
    smm = f32c(np.concatenate([np.asarray(inputs["pre_sm"]).reshape(-1),
                               np.asarray(inputs["post_sm"]).reshape(-1)])[None, :])
    base = {
        "dwt": dwt, "vecs": vecs, "smm": smm,
        "pwq": f32c(inputs["pw_q"]), "pwk": f32c(inputs["pw_k"]),
        "pwvT": f32c(np.asarray(inputs["pw_v"]).T), "wout": f32c(inputs["w_out"]),
    }
    in_maps = []
    for i in range(N_CORES):
        m = dict(base)
        m["xq"] = np.ascontiguousarray(xq_t[BPC * i:BPC * (i + 1)])
        m["xkv"] = np.ascontiguousarray(xkv_t[BPC * i:BPC * (i + 1)])
        in_maps.append(m)
    return in_maps


def kernel(**inputs):
    global last_results
    nc = _built()
    in_maps = make_in_maps(inputs)
    trace = os.environ.get("BASS_KERNEL_TRACE", "0") == "1"
    res = run_bass_kernel_spmd(nc, in_maps, core_ids=list(range(N_CORES)), trace=trace)
    last_results = res
    out = np.concatenate([res.results[i]["out"] for i in range(N_CORES)], axis=0)
    return out.astype(np.float32)


if __name__ == "__main__":
    import reference
    inputs = reference.setup_inputs()
    expected = np.asarray(reference.reference(**inputs))
    actual = kernel(**inputs)
    d = np.abs(actual - expected)
    print(f"absmax={d.max():.3e} scale={np.abs(expected).max():.3e} "
          f"rel={d.max() / np.abs(expected).max():.3e}")

